# revision 11
# baseline (speedup 1.0000x reference)
"""Trainium2 Bass kernel for the LUT-linear (embedding_lookup) problem.

Math: per_table[b,t] = sum_c lut[t,c] * prod_j (1 + s_{c,j} x_j)/2 with
x_0 = input[b, mask[2t]], x_1 = input[b, mask[2t+1]], K=2 (KK=4 corners).
Expanding the corner products (codes s in {-1,+1}):
    per_table = a_t + b_t x0 + c_t x1 + d_t x0 x1
    4a = w0+w1+w2+w3, 4b = -w0+w1-w2+w3, 4c = -w0-w1+w2+w3, 4d = w0-w1-w2+w3
out[b,o] = bias[o] + sum_{t in seg_o} per_table   (segments are 512 contiguous
tables per out-feature).

Device strategy (8 NeuronCores, table-sharded; input replicated):
  - per core: 32768 tables = 64 out-features. Gather x0/x1 columns with
    SWDGE dma_gather from input^T [512, 64] f32 (256B rows); each
    descriptor moves all 64 batch values of one input feature into SBUF
    tiles [128 part, W, 64] (partition = table mod 128, free = batch).
    Gathers are 1024 indices each (ucode ring limit), spread round-robin
    over 4 SWDGE queues so Q7 desc-gen and SDMA drain pipeline.
  - DVE computes y = x0*(b + d*x1) + c*x1 via broadcast-coefficient
    tensor_tensor ops, reduces tables along the free axis; the constant
    term a is reduced separately; partitions pair-sum to out-features via
    a tiny PE matmul with a 0/1 pairing matrix.
  - Host does only data-independent layout transforms (transpose, cast,
    permute, shard) and the final unshard.
  - Measured: ~204 us HW exec per core, rel err 2.2e-7 vs f32 reference.
"""

import numpy as np

NCORES = 8
B = 64
IN = 512
OUT = 512
T = IN * OUT
TC = T // NCORES          # tables per core = 32768
SEG = 512                 # tables per out-feature
OC = OUT // NCORES        # out-features per core = 64
NPART = 128
WT = TC // NPART          # tables per partition total = 256

# tuning knobs
NCHUNK = 8                # compute chunks per core
W = WT // NCHUNK          # tables per partition per chunk
TCHUNK = NPART * W        # tables per chunk
GIDX = 1024               # indices per dma_gather (ucode limit)
GSUB = TCHUNK // GIDX     # sub-gathers per compute chunk
GW = GIDX // NPART        # tables per partition per sub-gather
NQUEUES = 4

_CACHE = {}


def _build_program():
    import concourse.bacc as bacc
    import concourse.mybir as mybir
    from concourse import library_config
    from concourse.tile import TileContext

    f32 = mybir.dt.float32
    i16 = mybir.dt.int16
    Alu = mybir.AluOpType
    Axis = mybir.AxisListType

    S = TCHUNK // 16      # idx columns per chunk (16-partition wrap)

    nc = bacc.Bacc("TRN2", target_bir_lowering=False, debug=False,
                   num_devices=NCORES, num_swdge_queues=NQUEUES,
                   dynamic_dma_scratch_size=32768)

    input_t = nc.dram_tensor("input_t", [IN, B], f32, kind="ExternalInput")
    idx0_d = nc.dram_tensor("idx0", [NPART, NCHUNK * S], i16, kind="ExternalInput")
    idx1_d = nc.dram_tensor("idx1", [NPART, NCHUNK * S], i16, kind="ExternalInput")
    lutp_d = nc.dram_tensor("lutp", [NCHUNK, NPART, W * 4], f32, kind="ExternalInput")
    bias_d = nc.dram_tensor("bias_sh", [OC, 1], f32, kind="ExternalInput")
    pm_d = nc.dram_tensor("pm", [NPART, OC], f32, kind="ExternalInput")
    out_d = nc.dram_tensor("out_c", [OC, B], f32, kind="ExternalOutput")

    with TileContext(nc) as tc:
        nc.gpsimd.load_library(library_config.mlp)
        with (
            tc.tile_pool(name="idx", bufs=1) as idx_pool,
            tc.tile_pool(name="small", bufs=1) as small_pool,
            tc.tile_pool(name="lut", bufs=2) as lut_pool,
            tc.tile_pool(name="coef", bufs=2) as coef_pool,
            tc.tile_pool(name="x0", bufs=4) as x0_pool,
            tc.tile_pool(name="x1", bufs=4) as x1_pool,
            tc.tile_pool(name="m", bufs=3) as m_pool,
            tc.tile_pool(name="red", bufs=2) as red_pool,
            tc.tile_pool(name="psum", bufs=1, space="PSUM") as psum_pool,
        ):
            idx0_sb = idx_pool.tile([NPART, NCHUNK * S], i16, tag="idx0")
            idx1_sb = idx_pool.tile([NPART, NCHUNK * S], i16, tag="idx1")
            nc.sync.dma_start(idx0_sb[:], idx0_d[:])
            nc.sync.dma_start(idx1_sb[:], idx1_d[:])

            pm_sb = small_pool.tile([NPART, OC], f32, tag="pm")
            nc.sync.dma_start(pm_sb[:], pm_d[:])
            bias_sb = small_pool.tile([OC, 1], f32, tag="bias")
            nc.sync.dma_start(bias_sb[:], bias_d[:])

            partial = small_pool.tile([NPART, B], f32, tag="partial")
            apart = small_pool.tile([NPART, 1], f32, tag="apart")
            nc.vector.memset(partial[:], 0.0)
            nc.vector.memset(apart[:], 0.0)

            for c in range(NCHUNK):
                w4 = lut_pool.tile([NPART, W, 4], f32, tag="w4")
                nc.sync.dma_start(w4[:], lutp_d[c].rearrange("p (w k) -> p w k", k=4))

                # coefficient transform (values are 4x the true a,b,c,d;
                # folded back by the 0.25 scale at the end)
                ca = coef_pool.tile([NPART, W], f32, tag="ca")
                cb = coef_pool.tile([NPART, W], f32, tag="cb")
                cc = coef_pool.tile([NPART, W], f32, tag="cc")
                cd = coef_pool.tile([NPART, W], f32, tag="cd")
                t1 = coef_pool.tile([NPART, W], f32, tag="t1")
                t2 = coef_pool.tile([NPART, W], f32, tag="t2")
                nc.vector.tensor_tensor(t1[:], w4[:, :, 0], w4[:, :, 3], Alu.add)
                nc.vector.tensor_tensor(t2[:], w4[:, :, 1], w4[:, :, 2], Alu.add)
                nc.vector.tensor_tensor(ca[:], t1[:], t2[:], Alu.add)
                nc.vector.tensor_tensor(cd[:], t1[:], t2[:], Alu.subtract)
                nc.vector.tensor_tensor(t1[:], w4[:, :, 3], w4[:, :, 0], Alu.subtract)
                nc.vector.tensor_tensor(t2[:], w4[:, :, 1], w4[:, :, 2], Alu.subtract)
                nc.vector.tensor_tensor(cb[:], t1[:], t2[:], Alu.add)
                nc.vector.tensor_tensor(cc[:], t1[:], t2[:], Alu.subtract)

                GS = GIDX // 16   # idx columns per sub-gather
                x0 = x0_pool.tile([NPART, W, B], f32, tag="x0")
                x1 = x1_pool.tile([NPART, W, B], f32, tag="x1")
                for j in range(GSUB):
                    i0 = c * S + j * GS
                    q = (c * GSUB * 2 + 2 * j) % NQUEUES
                    nc.gpsimd.dma_gather(
                        x0[:, j * GW:(j + 1) * GW, :], input_t[:],
                        idx0_sb[:, i0:i0 + GS], GIDX, GIDX, B, queue_num=q)
                    nc.gpsimd.dma_gather(
                        x1[:, j * GW:(j + 1) * GW, :], input_t[:],
                        idx1_sb[:, i0:i0 + GS], GIDX, GIDX, B,
                        queue_num=(q + 1) % NQUEUES)

                # y = x0*(b + d*x1) + c*x1 (+ a via apart)
                u = m_pool.tile([NPART, W, B], f32, tag="u")
                bcb = cb[:].unsqueeze(2).broadcast_to([NPART, W, B])
                bcc = cc[:].unsqueeze(2).broadcast_to([NPART, W, B])
                bcd = cd[:].unsqueeze(2).broadcast_to([NPART, W, B])
                nc.vector.tensor_tensor(u[:], x1[:], bcd, Alu.mult)
                nc.vector.tensor_tensor(u[:], u[:], bcb, Alu.add)
                nc.vector.tensor_tensor(u[:], u[:], x0[:], Alu.mult)
                nc.vector.tensor_tensor(x1[:], x1[:], bcc, Alu.mult)
                nc.vector.tensor_tensor(x1[:], x1[:], u[:], Alu.add)

                red = red_pool.tile([NPART, B], f32, tag="red")
                nc.vector.tensor_reduce(
                    red[:], x1[:].transpose([0, 2, 1]), Axis.X, Alu.add)
                nc.vector.tensor_tensor(partial[:], partial[:], red[:], Alu.add)

                reda = red_pool.tile([NPART, 1], f32, tag="reda")
                nc.vector.tensor_reduce(reda[:], ca[:], Axis.X, Alu.add)
                nc.vector.tensor_tensor(apart[:], apart[:], reda[:], Alu.add)

            # total = partial + apart (per-partition broadcast along batch)
            nc.vector.tensor_scalar(partial[:], partial[:], apart[:], None, Alu.add)

            # pair-sum partitions to out-features: psum[o, b] = sum_p pm[p,o]*partial[p,b]
            ps = psum_pool.tile([OC, B], f32, tag="ps")
            nc.tensor.matmul(ps[:], pm_sb[:], partial[:], start=True, stop=True)

            out_sb = small_pool.tile([OC, B], f32, tag="out")
            nc.vector.tensor_scalar(out_sb[:], ps[:], 0.25, bias_sb[:], Alu.mult, Alu.add)
            nc.sync.dma_start(out_d[:], out_sb[:])

    nc.compile()
    return nc


def _host_prep(input, input_mask, lut_weights, bias):
    input_t = np.ascontiguousarray(input.T).astype(np.float32, copy=False)
    m0 = input_mask[0::2]
    m1 = input_mask[1::2]

    p = np.arange(NPART)
    c = np.arange(NCHUNK)
    w = np.arange(W)
    # core-local table index for (partition, chunk, within-partition slot)
    tau = ((p[:, None, None] // 2) * SEG + (p[:, None, None] % 2) * (SEG // 2)
           + c[None, :, None] * W + w[None, None, :])          # [128, NCHUNK, W]
    tau_cwp = np.ascontiguousarray(tau.transpose(1, 2, 0))     # [NCHUNK, W, 128]

    pm = np.zeros((NPART, OC), dtype=np.float32)
    pm[p, p // 2] = 1.0

    def wrap_idx(vals):  # [NCHUNK, W, 128] gather order -> dma_gather layout
        # wrap each GIDX-index sub-gather separately (16-partition wrap)
        wrapped = vals.reshape(NCHUNK * GSUB, GIDX // 16, 16).transpose(0, 2, 1)
        wrapped = np.tile(wrapped, (1, 8, 1))                  # [NCHUNK*GSUB, 128, GIDX//16]
        wrapped = wrapped.reshape(NCHUNK, GSUB, NPART, GIDX // 16)
        return np.ascontiguousarray(
            wrapped.transpose(2, 0, 1, 3).reshape(NPART, -1)).astype(np.int16)

    in_maps = []
    for core in range(NCORES):
        g = core * TC + tau_cwp                                # global tables
        lutp = lut_weights[core * TC + tau]                    # [128, NCHUNK, W, 4]
        lutp = np.ascontiguousarray(
            lutp.transpose(1, 0, 2, 3).reshape(NCHUNK, NPART, W * 4)
        ).astype(np.float32, copy=False)
        in_maps.append({
            "input_t": input_t,
            "idx0": wrap_idx(m0[g]),
            "idx1": wrap_idx(m1[g]),
            "lutp": lutp,
            "bias_sh": np.ascontiguousarray(
                bias[core * OC:(core + 1) * OC].reshape(OC, 1)
            ).astype(np.float32, copy=False),
            "pm": pm,
        })
    return in_maps


def get_program():
    if "nc" not in _CACHE:
        _CACHE["nc"] = _build_program()
    return _CACHE["nc"]


def run(input, input_mask, lut_weights, bias, trace=False):
    from concourse.bass_utils import run_bass_kernel_spmd

    nc = get_program()
    in_maps = _host_prep(np.asarray(input), np.asarray(input_mask),
                         np.asarray(lut_weights), np.asarray(bias))
    res = run_bass_kernel_spmd(nc, in_maps, list(range(NCORES)), trace=trace)
    out = np.concatenate([r["out_c"].T for r in res.results], axis=1)
    return out.astype(np.float32, copy=False), res


def kernel(input, input_mask, lut_weights, bias):
    out, _ = run(input, input_mask, lut_weights, bias)
    return out
